# revision 9
# baseline (speedup 1.0000x reference)
"""AttentionPool Trainium2 kernel.

Problem: x[B=8, S=4096, D=768] f32; att_v[768]; att_W[768, 768].
  y = tanh(x @ W); scores = y . v; w = softmax(scores over S); out = w . x  -> [B, D]

Sharding: pure data-parallel over batch B - one batch per NeuronCore, 8 cores,
no collectives.

Per-core pipeline, per 4-seq-tile group g (512 rows of x):
  1. HWDGE f32 load of 4 x tiles into a staging ring (sync queue)
  2. GpSimd: rounded-bf16 cast of the group (xs f32 -> xb bf16)
  3. XBAR dma transpose (scalar queue, SBUF->SBUF, off the HBM path):
     xb [128, 3072] -> xt [128, 24, 128] bf16 = per-chunk x^T
  4. DVE: fp8e4 cast xt -> xt8 (y-matmul operand)
  5. PE, per tile: 6 fp8 DoubleRow matmuls (3 k-pairs x {512, 256} cols):
     yps = x^T.T @ (64*W) accumulated in f32 psum
  6. ACT: t = tanh(yps / 64) -> bf16
  7. DVE: scores[:, q] = sum_e t*v  (scalar_tensor_tensor accum_out)
  8. ACT, per group: u = exp(scores) -> bf16 (|scores| < ~0.4, no max
     subtraction needed), accum_out -> Z partial column
  9. PE, per group, deferred 1 group: p[32q] += u_q.T @ xb_q - bf16 M=1
     matmuls into 4 accumulator rows in distinct PE col-groups.
Host: out = sum(p rows) / Z.

The PE runs only bf16/fp8 matmuls (no transposes, no fp32), keeping FWL on
and the HAM clock-gate warm.
"""

import sys

sys.path.insert(0, "/opt/trn_rl_repo")

import numpy as np

import concourse.bass as bass
import concourse.mybir as mybir
import concourse.tile as tile
from concourse.bass_utils import run_bass_kernel_spmd

P = 128
S = 4096
D = 768
NT = S // P  # 32 sequence tiles
DJ = D // P  # 6 contraction chunks
TPG = 4  # seq tiles per group
G = NT // TPG  # 8 groups
NPAIR = DJ // 2  # 3 DoubleRow k-pairs
NCORES = 8
SW = 64.0  # fp8 scale on W
FP8_ACT_CHUNKS = 20  # of 24 per-group fp8-cast chunks done on ACT (rest DVE)
USE_TTR = False  # tensor_tensor_reduce for scores (else scalar_tensor_tensor)

F32 = mybir.dt.float32
BF16 = mybir.dt.bfloat16
F8 = mybir.dt.float8e4
ACTF = mybir.ActivationFunctionType
DR = mybir.MatmulPerfMode.DoubleRow


def _build(split_waits: bool = True) -> bass.Bass:
    nc = bass.Bass()
    x_d = nc.declare_dram_parameter("x", [S, D], F32, isOutput=False)
    v_d = nc.declare_dram_parameter("att_v", [D], F32, isOutput=False)
    w_d = nc.declare_dram_parameter("att_W", [D, D], F32, isOutput=False)
    p_d = nc.declare_dram_parameter("out_p", [4, D], F32, isOutput=True)
    z_d = nc.declare_dram_parameter("out_z", [P, G], F32, isOutput=True)

    with tile.TileContext(nc) as tc:
        with (
            tc.tile_pool(name="singles", bufs=1) as singles,
            tc.tile_pool(name="stage", bufs=8) as stage_pool,
            tc.tile_pool(name="xb", bufs=4) as xb_pool,
            tc.tile_pool(name="xt", bufs=2) as xt_pool,
            tc.tile_pool(name="xt8", bufs=2) as xt8_pool,
            tc.tile_pool(name="tbuf", bufs=3) as t_pool,
            tc.tile_pool(name="scr", bufs=2) as scr_pool,
            tc.tile_pool(name="sc", bufs=3) as sc_pool,
            tc.tile_pool(name="ypsum", bufs=3, space="PSUM") as ypsum_pool,
            tc.tile_pool(name="ppsum", bufs=1, space="PSUM") as ppsum_pool,
        ):
            w_f32 = singles.tile([P, DJ, D], F32)
            w_f8 = singles.tile([P, DJ, D], F8)
            v_f32 = singles.tile([P, D], F32)
            v_bc = singles.tile([P, D], BF16)
            # per-group partial Z accumulators; host sums the values.
            zg = singles.tile([P, G], F32)
            # pooling accumulator psum: 4 col-group accumulator rows
            # (partitions 0/32/64/96), summed on the host. Memset once so the
            # final single-copy read of all 128 rows is well-defined.
            p_ps = ppsum_pool.tile([P, D], F32)
            nc.vector.memset(p_ps, 0.0)

            stage_tiles = {}
            xb_tiles = {}
            xt8_tiles = {}
            u_tiles = {}
            sc_tiles = {}

            def emit_params():
                # W f32 load on the sync ring (the x stream owns the scalar
                # ring); casts on ACT. Pair order matters: the first y
                # matmuls need slabs (0,1) first.
                for j in range(DJ):
                    nc.sync.dma_start(
                        out=w_f32[:, j, :], in_=w_d[j * P : (j + 1) * P, :]
                    )
                    nc.scalar.activation(
                        out=w_f8[:, j, :],
                        in_=w_f32[:, j, :],
                        func=ACTF.Copy,
                        scale=SW,
                    )
                nc.sync.dma_start(
                    out=v_f32, in_=v_d[:][None, :].to_broadcast([P, D])
                )
                nc.vector.tensor_copy(out=v_bc, in_=v_f32)

            def emit_load(g):
                xs = stage_pool.tile([P, TPG, D], F32, name="xs")
                nc.scalar.dma_start(
                    out=xs[:],
                    in_=x_d[g * TPG * P : (g + 1) * TPG * P, :].rearrange(
                        "(q p) d -> p q d", p=P
                    ),
                )
                stage_tiles[g] = xs

            xt_tiles = {}

            def emit_xform_a(g):
                # rounded bf16 cast (DVE 2x_2P mode) + XBAR transpose (sync
                # ring). Emitted a group ahead so the transpose is in flight
                # during group g-1's compute.
                xs = stage_tiles.pop(g)
                xb = xb_pool.tile([P, TPG, D], BF16, name="xb")
                nc.vector.tensor_copy(out=xb, in_=xs)
                xb_tiles[g] = xb
                xt = xt_pool.tile([P, TPG * DJ, P], BF16, name="xt")
                nc.sync.dma_start_transpose(out=xt[:], in_=xb[:])
                xt_tiles[g] = xt

            def emit_xform_b(g):
                # fp8 cast, split ACT/DVE. Emitted after group g-1's tanh so
                # its xt-wait never stalls ACT work that is already ready.
                xt = xt_tiles.pop(g)
                xt8 = xt8_pool.tile([P, TPG * DJ, P], F8, name="xt8")
                bs = FP8_ACT_CHUNKS
                nc.scalar.activation(
                    out=xt8[:, 0:bs, :], in_=xt[:, 0:bs, :], func=ACTF.Copy
                )
                nc.vector.tensor_copy(out=xt8[:, bs:, :], in_=xt[:, bs:, :])
                xt8_tiles[g] = xt8

            def emit_y(g, q):
                # 6 DoubleRow matmuls + tanh + scores for tile i = g*TPG + q
                xt8 = xt8_tiles[g]
                yps = ypsum_pool.tile([P, 1024], F32, name="yps")
                for jj in range(NPAIR):
                    c = q * DJ + 2 * jj
                    nc.tensor.matmul(
                        yps[:, 0:512],
                        lhsT=xt8[:, c : c + 2, :],
                        rhs=w_f8[:, 2 * jj : 2 * jj + 2, 0:512],
                        start=(jj == 0),
                        stop=(jj == NPAIR - 1),
                        perf_mode=DR,
                    )
                    nc.tensor.matmul(
                        yps[:, 512:D],
                        lhsT=xt8[:, c : c + 2, :],
                        rhs=w_f8[:, 2 * jj : 2 * jj + 2, 512:D],
                        start=(jj == 0),
                        stop=(jj == NPAIR - 1),
                        perf_mode=DR,
                    )
                t = t_pool.tile([P, D], BF16, name="t")
                nc.scalar.activation(
                    out=t, in_=yps[:, 0:D], func=ACTF.Tanh, scale=1.0 / SW
                )
                if q == 0:
                    sc_tiles[g] = sc_pool.tile([P, TPG], F32, name="sc4")
                dve_out = scr_pool.tile([P, D], BF16, name="dve_out")
                if USE_TTR:
                    nc.vector.tensor_tensor_reduce(
                        out=dve_out,
                        in0=t,
                        in1=v_bc,
                        scale=1.0,
                        scalar=0.0,
                        op0=mybir.AluOpType.mult,
                        op1=mybir.AluOpType.add,
                        accum_out=sc_tiles[g][:, q : q + 1],
                    )
                else:
                    nc.vector.scalar_tensor_tensor(
                        out=dve_out,
                        in0=t,
                        scalar=1.0,
                        in1=v_bc,
                        op0=mybir.AluOpType.mult,
                        op1=mybir.AluOpType.mult,
                        accum_out=sc_tiles[g][:, q : q + 1],
                    )

            def emit_exp(g):
                u4 = sc_pool.tile([P, TPG], BF16, name="u4")
                nc.scalar.activation(
                    out=u4,
                    in_=sc_tiles.pop(g),
                    func=ACTF.Exp,
                    accum_out=zg[:, g : g + 1],
                )
                u_tiles[g] = u4

            def emit_pool(g):
                # pooling for group g: 8 M=1 bf16 matmuls into 4 independent
                # accumulator rows (col-groups 0/32/64/96) so they run
                # concurrently in distinct 32-column strips of the PE array.
                # Deferred 1 group behind the main chain.
                u4 = u_tiles.pop(g)
                xb = xb_tiles.pop(g)
                for q in range(TPG):
                    u = u4[:, q : q + 1]
                    base = 32 * q
                    nc.tensor.matmul(
                        p_ps[base : base + 1, 0:512],
                        lhsT=u,
                        rhs=xb[:, q, 0:512],
                        start=(g == 0),
                        stop=(g == G - 1),
                        tile_position=(0, base),
                        skip_group_check=True,
                    )
                    nc.tensor.matmul(
                        p_ps[base : base + 1, 512:D],
                        lhsT=u,
                        rhs=xb[:, q, 512:D],
                        start=(g == 0),
                        stop=(g == G - 1),
                        tile_position=(0, base),
                        skip_group_check=True,
                    )

            # all loads upfront on the scalar ring: they run back-to-back
            # and never wait on compute. stage bufs=8 holds the full x.
            for g in range(G):
                emit_load(g)
            emit_params()
            emit_xform_a(0)
            emit_xform_b(0)
            for g in range(G):
                if g + 1 < G:
                    emit_xform_a(g + 1)
                for q in range(TPG):
                    emit_y(g, q)
                emit_exp(g)
                if g + 1 < G:
                    emit_xform_b(g + 1)
                if g >= 1:
                    emit_pool(g - 1)
            emit_pool(G - 1)

            # write out unnormalized p rows and the Z partials; the host sums
            # rows 0/32/64/96 of out_p and all of out_z
            p_sb = singles.tile([P, D], F32)
            nc.scalar.copy(out=p_sb, in_=p_ps)
            nc.sync.dma_start(out=p_d[:, :], in_=p_sb[0:97:32, :])
            nc.sync.dma_start(out=z_d[:, :], in_=zg)

    if split_waits:
        _split_excess_waits(nc)
    return nc


def _split_excess_waits(nc: bass.Bass) -> None:
    """Walrus accepts a single HW sync-wait per instruction (EventSemaphore
    excepted). Tile can attach more (data dep + DMA-lane reuse). Move all but
    one wait onto InstEventSemaphore(s) inserted just before, on the same
    engine — the sequencer executes waits in order, so semantics are
    unchanged."""
    fn = nc.m.functions[0]
    for blk in fn.blocks:
        insts = blk.instructions
        new_insts = []
        for inst in insts:
            si = inst.sync_info
            if (
                not isinstance(inst, mybir.InstEventSemaphore)
                and si is not None
                and len(si.on_wait) > 1
            ):
                waits = list(si.on_wait)
                for w in waits[:-1]:
                    ev = mybir.InstEventSemaphore(
                        name=nc.get_next_instruction_name(), ins=[], outs=[]
                    )
                    ev.engine = inst.engine
                    ev.sync_info = mybir.SyncInfo(on_wait=[w], on_update=[])
                    new_insts.append(ev)
                inst.sync_info = mybir.SyncInfo(
                    on_wait=waits[-1:], on_update=list(si.on_update)
                )
            new_insts.append(inst)
        blk.instructions = new_insts


_CACHE: dict = {}
LAST_RESULT = None


def _get_nc() -> bass.Bass:
    if "nc" not in _CACHE:
        _CACHE["nc"] = _build()
    return _CACHE["nc"]


def kernel(x: np.ndarray, att_v: np.ndarray, att_W: np.ndarray) -> np.ndarray:
    global LAST_RESULT
    assert x.shape == (NCORES, S, D), x.shape
    nc = _get_nc()
    in_maps = [
        {
            "x": np.ascontiguousarray(x[b], dtype=np.float32),
            "att_v": np.ascontiguousarray(att_v, dtype=np.float32),
            "att_W": np.ascontiguousarray(att_W, dtype=np.float32),
        }
        for b in range(NCORES)
    ]
    res = run_bass_kernel_spmd(nc, in_maps, core_ids=list(range(NCORES)))
    LAST_RESULT = res
    outs = []
    for b in range(NCORES):
        p = res.results[b]["out_p"].sum(axis=0, dtype=np.float64)
        z = res.results[b]["out_z"].sum(dtype=np.float64)
        outs.append(p / z)
    return np.stack(outs).astype(np.float32)
